# revision 12
# baseline (speedup 1.0000x reference)
"""Trainium2 Bass kernel: 4096x4096 fp32 'valid' cross-correlation with a 15x15
kernel, plus scalar bias.

Strategy (v3: fp8 DoubleRow, hi/lo weight planes)
-------------------------------------------------
- Shard the output 2x4 across 8 NeuronCores: 2 W-stripes of 2048 cols x 4
  H-bands of 1026 rows (tails trimmed on the host). Per core, 9 h-chunks of
  114 output rows: a banded-Toeplitz stationary (T[k, m] = wcol[k-m])
  contracts 128 input rows against 114 output rows; the W-shift for tap t is
  a free-dim offset into the moving tile.
- fp8e4m3 operands with perf_mode=DoubleRow (0.5 cycles/column, 2 MACs per
  cell). The weights are split w = w_hi + w_lo (both e4m3; residual ~1e-3
  relative), and the two DoubleRow interleave planes carry (w_hi_t, w_lo_t)
  for the SAME tap: the moving operand uses a zero-stride broadcast AP so
  both planes stream the same image columns. 15 passes per chunk at 0.5
  cyc/col vs v1's 15 bf16 passes at 1 cyc/col -> ~2x PE time, full weight
  precision.
- Accuracy (rel-err budget 2e-2): remaining device error is fp8 quantization
  of x. Its dominant component, mean(w) * boxsum15(x - xq), is computed
  exactly on the host via prefix sums and added to the downloaded output
  (with the bias) — the device never sees it. Residual error measures
  ~1.4e-2 rel in simulation.
- DMA rings: x loads on SP (nc.sync), output stores on ACT (nc.scalar).
"""

import numpy as np

H, W = 4096, 4096
KH, KW = 15, 15
HO, WO = H - KH + 1, W - KW + 1  # 4082, 4082
NCORES = 8
WSH, HSH = 2, 4          # core grid: 2 W-stripes x 4 H-bands
C = 2048                 # output cols per stripe
MCH = 114                # output rows per h-chunk (114 + 14 = 128 = K)
NCHUNK = 9               # chunks per band
B = NCHUNK * MCH         # output rows per band = 1026
BIN = B + KH - 1         # input rows per band = 1040
NBLK = C // 512          # 512-col psum blocks per chunk
CW = 2064                # moving-tile width (max col 1536+14+511 = 2061)
XSW = 2080               # per-core input dram width
XR_PAD = HSH * B + KH - 1   # padded input rows = 4118
XC_PAD = WSH * C + XSW - C  # padded input cols = 4128
NPASS = KW               # one pass per tap column; planes carry (hi, lo)

_CACHE = {}


def _bf16():
    import ml_dtypes
    return ml_dtypes.bfloat16


def _fp8():
    import ml_dtypes
    return ml_dtypes.float8_e4m3


def _enable_ldw_opt():
    """Flip walrus --enable-ldw-opt to true (dedupes identical consecutive
    weight loads)."""
    import concourse.bass_utils as bu
    if getattr(bu.run_command, "_ldw_patched", False):
        return
    orig = bu.run_command

    def patched(argv, **kw):
        argv = ["--enable-ldw-opt=true" if a == "--enable-ldw-opt=false" else a
                for a in argv]
        return orig(argv, **kw)

    patched._ldw_patched = True
    bu.run_command = patched


def _build_nc(reps: int = 1, hw_loop: bool = False, ldw_opt: bool = False,
              probe_same_w: bool = False,
              parts: tuple = ("in", "mm", "drain", "out")):
    import concourse.bacc as bacc
    import concourse.mybir as mybir
    from concourse.tile import TileContext

    if ldw_opt:
        _enable_ldw_opt()
    parts = set(parts)
    f32 = mybir.dt.float32
    bf16 = mybir.dt.bfloat16
    fp8 = mybir.dt.float8e4

    nc = bacc.Bacc("TRN2", debug=False, num_devices=NCORES)
    xs_d = nc.dram_tensor("xs", [BIN, XSW], fp8, kind="ExternalInput")
    wT_d = nc.dram_tensor("wT", [128, NPASS, 2, 128], fp8, kind="ExternalInput")
    ys_d = nc.dram_tensor("ys", [B, C], bf16, kind="ExternalOutput")

    with TileContext(nc) as tc:
        with (
            tc.tile_pool(name="xp", bufs=2) as xp,
            tc.tile_pool(name="wp", bufs=1) as wp,
            tc.tile_pool(name="op", bufs=3) as op,
            tc.tile_pool(name="pp", bufs=2, space="PSUM") as pp,
        ):
            w_t = wp.tile([128, NPASS, 2, 128], fp8)
            nc.sync.dma_start(w_t[:, :, :, :], wT_d[:, :, :, :])

            x_s = o_s = None
            if "mm" in parts and "in" not in parts:
                x_s = wp.tile([128, CW], fp8)
                nc.sync.dma_start(x_s[:, :], xs_d[0:128, 0:CW])
            if "out" in parts and "drain" not in parts:
                o_s = wp.tile([MCH, C], bf16)
                nc.vector.memset(o_s[:, :], 0.0)

            def rep_body(_i=None):
                for ci in range(NCHUNK):
                    m0 = ci * MCH
                    if "in" in parts:
                        x_b = xp.tile([128, CW], fp8, name="x_b")
                        nc.sync.dma_start(x_b[:, :], xs_d[m0:m0 + 128, 0:CW])
                    else:
                        x_b = x_s
                    if "drain" in parts:
                        o = op.tile([MCH, C], bf16, name="o")
                    else:
                        o = o_s
                    if "mm" in parts:
                        pss = [pp.tile([128, 512], f32, name=f"ps{b}")
                               for b in range(NBLK)]
                        for t in range(NPASS):
                            for blk in range(NBLK):
                                j0 = blk * 512 + t
                                rhs = (x_b[:, j0:j0 + 512]
                                       .unsqueeze(1)
                                       .broadcast_to([128, 2, 512]))
                                nc.tensor.matmul(
                                    pss[blk][:, :],
                                    w_t[:, 0 if probe_same_w else t, :, :],
                                    rhs,
                                    start=(t == 0),
                                    stop=(t == NPASS - 1),
                                    perf_mode=mybir.MatmulPerfMode.DoubleRow,
                                    skip_group_check=True,
                                )
                        if "drain" in parts:
                            for blk in range(NBLK):
                                nc.vector.tensor_scalar_add(
                                    o[:, blk * 512:(blk + 1) * 512],
                                    pss[blk][0:MCH, :],
                                    0.0,
                                )
                    if "out" in parts:
                        nc.scalar.dma_start(ys_d[m0:m0 + MCH, :], o[:, :])

            if hw_loop and reps > 1:
                tc.For_i_unrolled(0, reps, 1, rep_body, max_unroll=8)
            else:
                for _rep in range(reps):
                    rep_body()

    nc.compile()
    return nc


def _toeplitz(col: np.ndarray) -> np.ndarray:
    """T[k, m] = col[k-m] for 0 <= k-m < KH (m < MCH; cols MCH..127 zero)."""
    T = np.zeros((128, 128), dtype=np.float32)
    for di in range(KH):
        for m in range(MCH):
            T[m + di, m] = col[di]
    return T


def _weight_stack(w_hi: np.ndarray, w_lo: np.ndarray) -> np.ndarray:
    """wT[k, t, plane, m]: plane0 = Toeplitz(w_hi[:,t]), plane1 = w_lo."""
    wT = np.zeros((128, NPASS, 2, 128), dtype=np.float32)
    for t in range(NPASS):
        wT[:, t, 0, :] = _toeplitz(w_hi[:, t])
        wT[:, t, 1, :] = _toeplitz(w_lo[:, t])
    return wT


def _boxsum15(a: np.ndarray) -> np.ndarray:
    """Valid 15x15 box sum (fp64 prefix sums)."""
    c = np.cumsum(np.cumsum(a, axis=0, dtype=np.float64), axis=1)
    c = np.pad(c, ((1, 0), (1, 0)))
    return (c[KH:, KW:] - c[:-KH, KW:] - c[KH:, :-KW]
            + c[:-KH, :-KW]).astype(np.float32)


def _prepare_in_maps(x, weight, bias):
    bf16 = _bf16()
    fp8 = _fp8()
    x = np.ascontiguousarray(x, dtype=np.float32)
    w = np.asarray(weight, dtype=np.float32)
    bias_v = float(np.asarray(bias, dtype=np.float32).reshape(-1)[0])

    w_hi = w.astype(fp8).astype(np.float32)
    w_lo = (w - w_hi).astype(fp8).astype(np.float32)

    x_pad = np.zeros((XR_PAD, XC_PAD), dtype=np.float32)
    x_pad[:H, :W] = x
    xq_pad = x_pad.astype(fp8)

    # Host correction: mean(w) * boxsum15(x - xq) + bias, added to the
    # downloaded output in kernel() (device never sees it).
    x_lo = x_pad - xq_pad.astype(np.float32)
    corr = w.mean() * _boxsum15(x_lo) + bias_v

    wT = _weight_stack(w_hi, w_lo).astype(fp8)

    in_maps = []
    for core in range(NCORES):
        c, r = core // HSH, core % HSH
        xs = xq_pad[r * B:r * B + BIN, c * C:c * C + XSW]
        in_maps.append({"xs": np.ascontiguousarray(xs), "wT": wT})
    return in_maps, corr


def kernel(x: np.ndarray, weight: np.ndarray, bias: np.ndarray) -> np.ndarray:
    from concourse.bass_utils import run_bass_kernel_spmd

    if "nc" not in _CACHE:
        _CACHE["nc"] = _build_nc()
    nc = _CACHE["nc"]

    in_maps, corr = _prepare_in_maps(x, weight, bias)
    res = run_bass_kernel_spmd(nc, in_maps, core_ids=list(range(NCORES)))

    out = np.empty((HO, WO), dtype=np.float32)
    for core in range(NCORES):
        c, r = core // HSH, core % HSH
        r0, r1 = r * B, min(r * B + B, HO)
        c0, c1 = c * C, min(c * C + C, WO)
        ys = res.results[core]["ys"]
        out[r0:r1, c0:c1] = (ys[: r1 - r0, : c1 - c0].astype(np.float32)
                             + corr[r0:r1, c0:c1])
    return out


# revision 19
# speedup vs baseline: 1.4565x; 1.4565x over previous
"""Trainium2 Bass kernel: 4096x4096 fp32 'valid' cross-correlation with a 15x15
kernel, plus scalar bias.

Strategy (v3: fp8 DoubleRow, hi/lo weight planes)
-------------------------------------------------
- Shard the output 2x4 across 8 NeuronCores: 2 W-stripes of 2048 cols x 4
  H-bands of 1026 rows (tails trimmed on the host). Per core, 9 h-chunks of
  114 output rows: a banded-Toeplitz stationary (T[k, m] = wcol[k-m])
  contracts 128 input rows against 114 output rows; the W-shift for tap t is
  a free-dim offset into the moving tile.
- fp8e4m3 operands with perf_mode=DoubleRow (0.5 cycles/column, 2 MACs per
  cell). The moving tile carries two planes of the same image rows at
  column shifts (0, +8), so one pass computes TWO taps (t, t+8). The
  weights are split w = w_hi + w_lo (both e4m3; residual ~1e-3 relative):
  15 passes = 7 hi-pairs + 7 lo-pairs + single hi7 cover 29 of the 30
  tap-slots; the w_lo column-7 slot is folded into the host correction.
  (A zero-stride broadcast moving AP would allow same-shift (hi,lo) pairs
  with no plane copy, but measures at 1.0 cyc/col — the degenerate stride
  defeats DoubleRow's double pumping. The +8-shift plane copy streams at
  the full 0.5 cyc/col rate.)
- Accuracy (rel-err budget 2e-2): remaining device error is fp8 quantization
  of x. Its dominant component, mean(w) * boxsum15(x - xq), is computed
  exactly on the host via prefix sums and added to the downloaded output
  (with the bias) — the device never sees it. Residual error measures
  ~1.4e-2 rel in simulation.
- DMA rings: x loads on SP (nc.sync), output stores on ACT (nc.scalar).
"""

import numpy as np

H, W = 4096, 4096
KH, KW = 15, 15
HO, WO = H - KH + 1, W - KW + 1  # 4082, 4082
NCORES = 8
WSH, HSH = 2, 4          # core grid: 2 W-stripes x 4 H-bands
C = 2048                 # output cols per stripe
MCH = 114                # output rows per h-chunk (114 + 14 = 128 = K)
NCHUNK = 9               # chunks per band
B = NCHUNK * MCH         # output rows per band = 1026
BIN = B + KH - 1         # input rows per band = 1040
NBLK = C // 512          # 512-col psum blocks per chunk
CW = 2064                # moving-tile width (max col 1536+14+511 = 2061)
XSW = 2080               # per-core input dram width
XR_PAD = HSH * B + KH - 1   # padded input rows = 4118
XC_PAD = WSH * C + XSW - C  # padded input cols = 4128
NPASS = 15
# pass p -> (base column shift, plane0 tap source, plane1 tap source)
# plane1 reads the +8-shifted data plane, so it covers tap base+8.
PASS_TABLE = (
    [(t, ("hi", t), ("hi", t + 8)) for t in range(7)]
    + [(t, ("lo", t), ("lo", t + 8)) for t in range(7)]
    + [(7, ("hi", 7), None)]
)

_CACHE = {}


def _bf16():
    import ml_dtypes
    return ml_dtypes.bfloat16


def _fp8():
    import ml_dtypes
    return ml_dtypes.float8_e4m3


def _enable_ldw_opt():
    """Flip walrus --enable-ldw-opt to true (dedupes identical consecutive
    weight loads)."""
    import concourse.bass_utils as bu
    if getattr(bu.run_command, "_ldw_patched", False):
        return
    orig = bu.run_command

    def patched(argv, **kw):
        argv = ["--enable-ldw-opt=true" if a == "--enable-ldw-opt=false" else a
                for a in argv]
        return orig(argv, **kw)

    patched._ldw_patched = True
    bu.run_command = patched


def _build_nc(reps: int = 1, hw_loop: bool = False, ldw_opt: bool = False,
              probe_same_w: bool = False,
              parts: tuple = ("in", "mm", "drain", "out")):
    import concourse.bacc as bacc
    import concourse.mybir as mybir
    from concourse.tile import TileContext

    if ldw_opt:
        _enable_ldw_opt()
    parts = set(parts)
    f32 = mybir.dt.float32
    bf16 = mybir.dt.bfloat16
    fp8 = mybir.dt.float8e4

    nc = bacc.Bacc("TRN2", debug=False, num_devices=NCORES)
    xs_d = nc.dram_tensor("xs", [BIN, XSW], fp8, kind="ExternalInput")
    wT_d = nc.dram_tensor("wT", [128, NPASS, 2, 128], fp8, kind="ExternalInput")
    ys_d = nc.dram_tensor("ys", [B, C], bf16, kind="ExternalOutput")

    with TileContext(nc) as tc:
        with (
            tc.tile_pool(name="xp", bufs=2) as xp,
            tc.tile_pool(name="wp", bufs=1) as wp,
            tc.tile_pool(name="op", bufs=3) as op,
            tc.tile_pool(name="pp", bufs=2, space="PSUM") as pp,
        ):
            w_t = wp.tile([128, NPASS, 2, 128], fp8)
            nc.sync.dma_start(w_t[:, :, :, :], wT_d[:, :, :, :])

            x_s = o_s = None
            if "mm" in parts and "in" not in parts:
                x_s = wp.tile([128, 2, CW], fp8)
                nc.sync.dma_start(x_s[:, 0, :], xs_d[0:128, 0:CW])
                nc.sync.dma_start(x_s[:, 1, :], xs_d[0:128, 8:8 + CW])
            if "out" in parts and "drain" not in parts:
                o_s = wp.tile([MCH, C], bf16)
                nc.vector.memset(o_s[:, :], 0.0)

            def rep_body(_i=None):
                for ci in range(NCHUNK):
                    m0 = ci * MCH
                    if "in" in parts:
                        x_b = xp.tile([128, 2, CW], fp8, name="x_b")
                        nc.sync.dma_start(x_b[:, 0, :],
                                          xs_d[m0:m0 + 128, 0:CW])
                        nc.sync.dma_start(x_b[:, 1, :],
                                          xs_d[m0:m0 + 128, 8:8 + CW])
                    else:
                        x_b = x_s
                    if "drain" in parts:
                        o = op.tile([MCH, C], bf16, name="o")
                    else:
                        o = o_s
                    if "mm" in parts:
                        pss = [pp.tile([128, 512], f32, name=f"ps{b}")
                               for b in range(NBLK)]
                        for p, (base, _s0, _s1) in enumerate(PASS_TABLE):
                            for blk in range(NBLK):
                                j0 = blk * 512 + base
                                nc.tensor.matmul(
                                    pss[blk][:, :],
                                    w_t[:, 0 if probe_same_w else p, :, :],
                                    x_b[:, :, j0:j0 + 512],
                                    start=(p == 0),
                                    stop=(p == NPASS - 1),
                                    perf_mode=mybir.MatmulPerfMode.DoubleRow,
                                    skip_group_check=True,
                                )
                        if "drain" in parts:
                            for blk in range(NBLK):
                                nc.vector.tensor_scalar_add(
                                    o[:, blk * 512:(blk + 1) * 512],
                                    pss[blk][0:MCH, :],
                                    0.0,
                                )
                    if "out" in parts:
                        nc.scalar.dma_start(ys_d[m0:m0 + MCH, :], o[:, :])

            if hw_loop and reps > 1:
                tc.For_i_unrolled(0, reps, 1, rep_body, max_unroll=8)
            else:
                for _rep in range(reps):
                    rep_body()

    nc.compile()
    return nc


def _toeplitz(col: np.ndarray) -> np.ndarray:
    """T[k, m] = col[k-m] for 0 <= k-m < KH (m < MCH; cols MCH..127 zero)."""
    T = np.zeros((128, 128), dtype=np.float32)
    for di in range(KH):
        for m in range(MCH):
            T[m + di, m] = col[di]
    return T


def _weight_stack(w_hi: np.ndarray, w_lo: np.ndarray) -> np.ndarray:
    """wT[k, pass, plane, m] per PASS_TABLE."""
    src = {"hi": w_hi, "lo": w_lo}
    wT = np.zeros((128, NPASS, 2, 128), dtype=np.float32)
    for p, (_base, s0, s1) in enumerate(PASS_TABLE):
        for plane, s in enumerate((s0, s1)):
            if s is not None:
                wT[:, p, plane, :] = _toeplitz(src[s[0]][:, s[1]])
    return wT


def _boxsum15(a: np.ndarray) -> np.ndarray:
    """Valid 15x15 box sum (fp64 prefix sums)."""
    c = np.cumsum(np.cumsum(a, axis=0, dtype=np.float64), axis=1)
    c = np.pad(c, ((1, 0), (1, 0)))
    return (c[KH:, KW:] - c[:-KH, KW:] - c[KH:, :-KW]
            + c[:-KH, :-KW]).astype(np.float32)


def _prepare_in_maps(x, weight, bias):
    bf16 = _bf16()
    fp8 = _fp8()
    x = np.ascontiguousarray(x, dtype=np.float32)
    w = np.asarray(weight, dtype=np.float32)
    bias_v = float(np.asarray(bias, dtype=np.float32).reshape(-1)[0])

    w_hi = w.astype(fp8).astype(np.float32)
    w_lo = (w - w_hi).astype(fp8).astype(np.float32)

    x_pad = np.zeros((XR_PAD, XC_PAD), dtype=np.float32)
    x_pad[:H, :W] = x
    xq_pad = x_pad.astype(fp8)

    # Host correction: mean(w) * boxsum15(x - xq) + exact col-7 w_lo
    # residual (vertical 1D conv over xq) + bias, added to the downloaded
    # output in kernel() (device never sees it).
    xq_f = xq_pad.astype(np.float32)
    x_lo = x_pad - xq_f
    corr = w.mean() * _boxsum15(x_lo) + bias_v
    lam7 = (w[:, 7] - w_hi[:, 7]).astype(np.float32)
    ho_pad, wo_pad = corr.shape
    for di in range(KH):
        if lam7[di] != 0.0:
            corr += lam7[di] * xq_f[di:di + ho_pad, 7:7 + wo_pad]

    wT = _weight_stack(w_hi, w_lo).astype(fp8)

    in_maps = []
    for core in range(NCORES):
        c, r = core // HSH, core % HSH
        xs = xq_pad[r * B:r * B + BIN, c * C:c * C + XSW]
        in_maps.append({"xs": np.ascontiguousarray(xs), "wT": wT})
    return in_maps, corr


def kernel(x: np.ndarray, weight: np.ndarray, bias: np.ndarray) -> np.ndarray:
    from concourse.bass_utils import run_bass_kernel_spmd

    if "nc" not in _CACHE:
        _CACHE["nc"] = _build_nc()
    nc = _CACHE["nc"]

    in_maps, corr = _prepare_in_maps(x, weight, bias)
    res = run_bass_kernel_spmd(nc, in_maps, core_ids=list(range(NCORES)))

    out = np.empty((HO, WO), dtype=np.float32)
    for core in range(NCORES):
        c, r = core // HSH, core % HSH
        r0, r1 = r * B, min(r * B + B, HO)
        c0, c1 = c * C, min(c * C + C, WO)
        ys = res.results[core]["ys"]
        out[r0:r1, c0:c1] = (ys[: r1 - r0, : c1 - c0].astype(np.float32)
                             + corr[r0:r1, c0:c1])
    return out
